# revision 2
# baseline (speedup 1.0000x reference)
"""Trainium2 Bass kernel for the DifferentOptionsPolicyNetwork problem.

Reference computation (per sample b with option o = option[b]):
    x  = relu(state[b] @ W1[o])          # [H=512]
    x  = relu(x @ W2[o])                 # [K=64]
    mean    = x @ Wm[o] + bm[o]          # [A=32]
    log_std = clip(x @ Ws[o] + bs[o], -20, 2)

Strategy: expert-parallel across the 8 NeuronCores. Samples are grouped by
option on the host (free); core o receives option o's weights plus its
samples (transposed, zero-padded to NPAD columns). Every device-side matmul
is then a plain dense matmul:
    h1T[H, n] = W1[o].T-free form:   h1T = lhsT(W1 chunk).T @ xT chunk
    h2T[K, n] accumulated over 4 H-chunks
    outT[2A, n] = [Wm|Ws ; bm|bs].T @ [h2T ; ones]   (bias folded via ones row)
Outputs come back as [A, NPAD] per core and are scattered/transposed back on
the host.
"""

import os
import sys
import types

import numpy as np

B, I, O, H, A = 2048, 256, 8, 512, 32
K = H // O  # 64
NPAD = 384  # per-core sample capacity; Binom(2048, 1/8) exceeds this w.p. ~1e-17
N_CORES = 8
LOG_STD_MIN, LOG_STD_MAX = -20.0, 2.0


def _ensure_axon_hooks_shim():
    # concourse.bass_utils imports antenv.axon_hooks when BASS_TRACE is set;
    # some images lack that module. Provide a minimal shim so tracing degrades
    # instead of crashing.
    try:
        import antenv.axon_hooks  # noqa: F401
        return
    except ImportError:
        pass
    try:
        import antenv
    except ImportError:
        return
    mod = types.ModuleType("antenv.axon_hooks")
    mod._hook = None
    mod.set_axon_ntff_profile_hook = lambda h: setattr(mod, "_hook", h)
    mod.get_axon_ntff_profile_hook = lambda: mod._hook
    sys.modules["antenv.axon_hooks"] = mod
    antenv.axon_hooks = mod


_cached_nc = None
last_run = None  # BassKernelResults of the most recent kernel() call


def _build_nc():
    import concourse.bass as bass
    import concourse.mybir as mybir
    import concourse.tile as tile
    from concourse import bacc

    f32 = mybir.dt.float32
    Act = mybir.ActivationFunctionType
    Alu = mybir.AluOpType

    nc = bacc.Bacc("TRN2", target_bir_lowering=False, debug=False)

    xt = nc.dram_tensor("xt", [2, 128, NPAD], f32, kind="ExternalInput")
    w1 = nc.dram_tensor("w1", [2, 128, H], f32, kind="ExternalInput")
    w2p = nc.dram_tensor("w2p", [128, 4, K], f32, kind="ExternalInput")
    sm = nc.dram_tensor("sm", [K + 1, 2 * A], f32, kind="ExternalInput")
    meanT = nc.dram_tensor("meanT", [A, NPAD], f32, kind="ExternalOutput")
    lsT = nc.dram_tensor("lsT", [A, NPAD], f32, kind="ExternalOutput")

    with tile.TileContext(nc) as tc:
        with (
            tc.tile_pool(name="sb", bufs=1) as pool,
            tc.tile_pool(name="ps", bufs=1, space=bass.MemorySpace.PSUM) as psum,
        ):
            w1_sb = [pool.tile([128, H], f32, tag=f"w1_{c}", name=f"w1_{c}") for c in range(2)]
            x_sb = [pool.tile([128, NPAD], f32, tag=f"x_{c}", name=f"x_{c}") for c in range(2)]
            for c in range(2):
                nc.sync.dma_start(w1_sb[c][:], w1[c])
                nc.sync.dma_start(x_sb[c][:], xt[c])
            w2_sb = pool.tile([128, 4, K], f32, tag="w2")
            nc.sync.dma_start(w2_sb[:], w2p[:])
            sm_sb = pool.tile([K + 1, 2 * A], f32, tag="sm")
            nc.sync.dma_start(sm_sb[:], sm[:])

            # h2x rows 0..K-1 = relu(h2T); row K = ones (bias lane for stage 3)
            h2x = pool.tile([K + 1, NPAD], f32, tag="h2x")
            nc.vector.memset(h2x[K : K + 1, :], 1.0)

            h1 = pool.tile([128, 4, NPAD], f32, tag="h1")
            h1_ps = [psum.tile([128, NPAD], f32, tag=f"h1ps_{h}", name=f"h1ps_{h}") for h in range(4)]

            # Stage 1: h1T[h*128:(h+1)*128, :] = relu(sum_c W1[c,:,hs].T @ xT[c])
            for h in range(4):
                for c in range(2):
                    nc.tensor.matmul(
                        h1_ps[h][:],
                        w1_sb[c][:, h * 128 : (h + 1) * 128],
                        x_sb[c][:],
                        start=(c == 0),
                        stop=(c == 1),
                    )
                if h % 2 == 0:
                    nc.scalar.activation(h1[:, h, :], h1_ps[h][:], Act.Relu)
                else:
                    nc.vector.tensor_scalar_max(h1[:, h, :], h1_ps[h][:], 0.0)

            # Stage 2: h2T = relu(sum_c W2[c].T @ h1[c])
            h2_ps = psum.tile([K, NPAD], f32, tag="h2ps")
            for c in range(4):
                nc.tensor.matmul(
                    h2_ps[:],
                    w2_sb[:, c, :],
                    h1[:, c, :],
                    start=(c == 0),
                    stop=(c == 3),
                )
            nc.scalar.activation(h2x[0:K, :], h2_ps[:], Act.Relu)

            # Stage 3: [meanT; lsT] = [Wm|Ws ; bm|bs].T @ [h2T ; ones]
            out_ps = psum.tile([2 * A, NPAD], f32, tag="outps")
            nc.tensor.matmul(out_ps[:], sm_sb[:], h2x[:], start=True, stop=True)

            mean_sb = pool.tile([A, NPAD], f32, tag="mean")
            nc.scalar.activation(mean_sb[:], out_ps[0:A, :], Act.Copy)
            ls_sb = pool.tile([A, NPAD], f32, tag="ls")
            nc.vector.tensor_scalar(
                out=ls_sb[:],
                in0=out_ps[A : 2 * A, :],
                scalar1=LOG_STD_MAX,
                scalar2=LOG_STD_MIN,
                op0=Alu.min,
                op1=Alu.max,
            )
            nc.sync.dma_start(meanT[:], mean_sb[:])
            nc.sync.dma_start(lsT[:], ls_sb[:])

    nc.compile()
    return nc


def _numpy_fallback(state, W1, W2, Wm, Ws, bm, bs, opt):
    # Only used if some option has more than NPAD samples (never happens for
    # randint-distributed options at B=2048).
    x = np.maximum(np.einsum("bi,bih->bh", state, W1[opt]), 0.0)
    x = np.maximum(np.einsum("bh,bhk->bk", x, W2[opt]), 0.0)
    mean = np.einsum("bk,bka->ba", x, Wm[opt]) + bm[opt]
    ls = np.einsum("bk,bka->ba", x, Ws[opt]) + bs[opt]
    return mean.astype(np.float32), np.clip(ls, LOG_STD_MIN, LOG_STD_MAX).astype(np.float32)


def kernel(state, W1, W2, Wm, Ws, bm, bs, option):
    global _cached_nc, last_run
    _ensure_axon_hooks_shim()
    from concourse.bass_utils import run_bass_kernel_spmd

    state = np.ascontiguousarray(np.asarray(state, dtype=np.float32))
    W1 = np.asarray(W1, dtype=np.float32)
    W2 = np.asarray(W2, dtype=np.float32)
    Wm = np.asarray(Wm, dtype=np.float32)
    Ws = np.asarray(Ws, dtype=np.float32)
    bm = np.asarray(bm, dtype=np.float32)
    bs = np.asarray(bs, dtype=np.float32)
    opt = np.asarray(option).astype(np.int32)

    idx = [np.nonzero(opt == o)[0] for o in range(O)]
    if max(len(ix) for ix in idx) > NPAD:
        return _numpy_fallback(state, W1, W2, Wm, Ws, bm, bs, opt)

    in_maps = []
    for o in range(O):
        ix = idx[o]
        xT = np.zeros((2, 128, NPAD), np.float32)
        xT.reshape(256, NPAD)[:, : len(ix)] = state[ix].T
        smalls = np.empty((K + 1, 2 * A), np.float32)
        smalls[:K, :A] = Wm[o]
        smalls[:K, A:] = Ws[o]
        smalls[K, :A] = bm[o]
        smalls[K, A:] = bs[o]
        in_maps.append(
            {
                "xt": xT,
                "w1": np.ascontiguousarray(W1[o].reshape(2, 128, H)),
                "w2p": np.ascontiguousarray(
                    W2[o].reshape(4, 128, K).transpose(1, 0, 2)
                ),
                "sm": smalls,
            }
        )

    if _cached_nc is None:
        _cached_nc = _build_nc()

    last_run = run_bass_kernel_spmd(
        _cached_nc, in_maps, core_ids=list(range(N_CORES))
    )

    mean = np.empty((B, A), np.float32)
    log_std = np.empty((B, A), np.float32)
    for o in range(O):
        ix = idx[o]
        mean[ix] = last_run.results[o]["meanT"][:, : len(ix)].T
        log_std[ix] = last_run.results[o]["lsT"][:, : len(ix)].T
    return mean, log_std
